# revision 14
# baseline (speedup 1.0000x reference)
"""Trainium2 Bass kernel for a 2-layer GRU autoencoder RNN (B=256, T=1024, H=128).

Strategy
--------
- Data-parallel over batch: B=256 -> 32 rows per NeuronCore, 8 cores.
  The output is a scalar sum-of-squares loss, so each core computes a
  partial loss and the host sums the 8 partials. No collectives.
- The linear "dynn" head (x_t) is folded into composed weight matrices:
  the recurrent state is just (h0, h1); x_t is never materialized.
  u-dependent terms enter each step via a [u_t; u_{t-1}; 1] K=33 operand.
- Layout: features on partitions, batch (32) on the free dim.  Gate
  matmuls keep weights stationary ([K,128] lhsT slices of one packed
  SBUF tile) and stream the [128,32] state as the moving operand.
- The critical chain per GRU cell is sigmoid(r) -> r*an_h -> +an_i ->
  tanh -> zc*n -> gate matmuls; everything else (sigmoid(z), 1-z, z*h,
  the h=z*h+zc*n add, the readout MLP and the loss) is scheduled into
  the chain's idle engine slots.  The on-chain matmuls consume the
  un-summed (z*h, zc*n) halves so the DVE add stays off-chain; PN PSUM
  banks ping-pong by step parity to keep WAR event-semaphores out of
  the DVE FIFO.
- The readout MLP ("menn", with x_t folded in) and the loss are
  evaluated in 16-step batches (hidden matmuls, relu, output matmuls +
  Square-with-accum_out spread over three consecutive steps).
"""

import os
import sys
import numpy as np

sys.path.insert(0, "/opt/trn_rl_repo")

import ml_dtypes

BF16 = ml_dtypes.bfloat16

# problem constants (hardcoded per instructions)
B, T = 256, 1024
U, Z, Y, H = 16, 16, 16, 128
NCORES = 8
BL = B // NCORES  # 32 batch rows per core
NBUF = 32         # ring depth for [h|n] slots (32 so 16-step menn batches
                  # hit contiguous slot ranges)
MK = 16           # menn/loss batch size in steps


def _compose_host(inp):
    """All O(weight)-sized host-side algebra. Returns dict of np arrays."""
    f32 = np.float32
    Wih0, Whh0 = inp["Wih0"].astype(f32), inp["Whh0"].astype(f32)
    Wih1, Whh1 = inp["Wih1"].astype(f32), inp["Whh1"].astype(f32)
    dW1, db1 = inp["dW1"].astype(f32), inp["db1"].astype(f32)
    dW2, db2 = inp["dW2"].astype(f32), inp["db2"].astype(f32)
    mW1, mb1 = inp["mW1"].astype(f32), inp["mb1"].astype(f32)
    mW2, mb2 = inp["mW2"].astype(f32), inp["mb2"].astype(f32)
    mW3, mb3 = inp["mW3"].astype(f32), inp["mb3"].astype(f32)

    Wih0u, Wih0x = Wih0[:, :U], Wih0[:, U:]
    dW1u, dW1h = dW1[:, :U], dW1[:, U:]
    dWc = dW2 @ dW1h            # [16,128]
    dWpc = dW2 @ dW1u           # [16,16]
    cbias = db1 @ dW2.T + db2   # [16]

    W0x_eff = Wih0x @ dWc       # [384,128]
    W0upc = Wih0x @ dWpc        # [384,16]
    g0const = Wih0x @ cbias     # [384]

    mW1x, mW1h = mW1[:, :Z], mW1[:, Z:]
    mW1c = mW1x @ dWc           # [128,128]
    mWu = mW1x @ dWpc           # [128,16]
    mbias = mW1x @ cbias + mb1  # [128]
    mW32 = mW3 @ mW2            # [16,128]
    ybias = mW3 @ mb2 + mb3     # [16]

    # --- pack all stationary (lhsT) weights into one [128, ncols] tile ---
    # Each entry is lhsT = W.T with K on partitions (rows), M on columns.
    # u2-gate weights have K=33: rows 0:16 u_t part, 16:32 u_{t-1} part,
    # row 32 the constant (ones row of u2).
    slices = {}
    cols = []
    off = 0

    def add(name, mat_t):  # mat_t: [K, M] fp32
        nonlocal off
        k, m = mat_t.shape
        slices[name] = (off, k, m)
        cols.append(mat_t)
        off += m

    for g, sl in (("r", slice(0, H)), ("z", slice(H, 2 * H)),
                  ("n", slice(2 * H, 3 * H))):
        w_u2 = np.zeros((33, H), f32)
        w_u2[0:16] = Wih0u[sl].T
        w_u2[16:32] = W0upc[sl].T
        w_u2[32] = g0const[sl]
        add(f"u2_{g}", w_u2)
        add(f"whh0_{g}", Whh0[sl].T)        # [128,128]
        add(f"w0x_{g}", W0x_eff[sl].T)      # [128,128]
        add(f"wih1_{g}", Wih1[sl].T)        # [128,128]
        add(f"whh1_{g}", Whh1[sl].T)        # [128,128]
    add("mw1h", mW1h.T)                     # [128,128]
    add("mw1c", mW1c.T)                     # [128,128]
    add("mwu", mWu.T)                       # [16,128]
    add("mw32", mW32.T)                     # [128,16]
    negi = np.zeros((Y + 1, Y), f32)        # [-I; ybias] against [y; 1]
    negi[0:Y] = -np.eye(Y, dtype=f32)
    negi[Y] = ybias
    add("negI", negi)                       # [17,16]
    add("I128", np.eye(H, dtype=f32))       # [128,128]

    wpack = np.zeros((128, off), f32)
    o2 = 0
    for mat in cols:
        k, m = mat.shape
        wpack[:k, o2:o2 + m] = mat
        o2 += m

    return dict(wpack=wpack, slices=slices, mbias=mbias, ybias=ybias)


def _prep_core_inputs(inp, comp):
    """Per-core input arrays for the NEFF. Layout transforms only."""
    u = np.asarray(inp["u"], np.float32)    # [B, U, T]
    y = np.asarray(inp["y"], np.float32)    # [B, Y, T]
    h0 = np.asarray(inp["h0"], np.float32)  # [2, B, H]

    in_maps = []
    for c in range(NCORES):
        bs = slice(c * BL, (c + 1) * BL)
        uc = u[bs]                           # [BL, U, T]
        yc = y[bs]
        # u2: [33, T*BL]; column t*BL+b
        u2 = np.zeros((33, T * BL), np.float32)
        ut = np.transpose(uc, (1, 2, 0)).reshape(U, T * BL)   # [U, T*BL]
        u2[0:16] = ut
        u2[16:32, BL:] = ut[:, :-BL]        # u_{t-1}; zeros at t=0
        u2[32, BL:] = 1.0                   # const row; zero at t=0
        ysb = np.ones((Y + 1, T * BL), np.float32)            # [17, T*BL]
        ysb[0:Y] = np.transpose(yc, (1, 2, 0)).reshape(Y, T * BL)
        in_maps.append({
            "u2": u2.astype(BF16),
            "ysb": ysb.astype(BF16),
            "wpack": comp["wpack"].astype(BF16),
            "h0T": np.ascontiguousarray(h0[0, bs].T).astype(BF16),  # [128,BL]
            "h1T": np.ascontiguousarray(h0[1, bs].T).astype(BF16),
            "mbias": comp["mbias"].reshape(H, 1).astype(np.float32),
            "ybias": comp["ybias"].reshape(Y, 1).astype(np.float32),
        })
    return in_maps


def build_graph(slices, t_steps=T):
    """Build the Bass/Tile graph (one core's program; SPMD across 8)."""
    import concourse.bass as bass  # noqa
    import concourse.mybir as mybir
    import concourse.tile as tile
    from concourse import bacc
    from concourse.tile_rust import add_dep_helper

    f32 = mybir.dt.float32
    bf16 = mybir.dt.bfloat16
    AF = mybir.ActivationFunctionType

    nc = bacc.Bacc()
    wcols = max(o + m for (o, k, m) in slices.values())
    u2_d = nc.declare_dram_parameter("u2", [33, T * BL], bf16, isOutput=False)
    y_d = nc.declare_dram_parameter("ysb", [Y + 1, T * BL], bf16,
                                isOutput=False)
    w_d = nc.declare_dram_parameter("wpack", [128, wcols], bf16, isOutput=False)
    h0_d = nc.declare_dram_parameter("h0T", [H, BL], bf16, isOutput=False)
    h1_d = nc.declare_dram_parameter("h1T", [H, BL], bf16, isOutput=False)
    mb_d = nc.declare_dram_parameter("mbias", [H, 1], f32, isOutput=False)
    yb_d = nc.declare_dram_parameter("ybias", [Y, 1], f32, isOutput=False)
    out_d = nc.declare_dram_parameter("out", [Y, 2 * (T // MK)], f32,
                                      isOutput=True)

    with tile.TileContext(nc) as tc:
        # ---- persistent SBUF ----
        with (
            tc.tile_pool(name="resident", bufs=1) as rp,
            tc.tile_pool(name="sg", bufs=4) as sgp,
            tc.tile_pool(name="small", bufs=4) as smp,
            tc.tile_pool(name="pg", bufs=1, space="PSUM") as pgp,
            tc.tile_pool(name="pn", bufs=1, space="PSUM") as pnp,
            tc.tile_pool(name="pm", bufs=1, space="PSUM") as pmp,
        ):
            UT = rp.tile([33, T * BL], bf16)
            YT = rp.tile([Y + 1, T * BL], bf16)
            WT = rp.tile([128, wcols], bf16)
            MB = rp.tile([H, 1], f32)
            YB = rp.tile([Y, 1], f32)
            R0 = rp.tile([128, NBUF * 2 * BL], bf16)
            R1 = rp.tile([128, NBUF * 2 * BL], bf16)
            LOSS = rp.tile([Y, 2 * (T // MK)], f32)

            ib = (NBUF - 1) * 2 * BL  # slot(-1) h-half
            # small/critical tensors first, then the big streams in
            # chunks so early steps don't wait for the full transfer
            nc.sync.dma_start(WT[:], w_d[:])
            nc.sync.dma_start(R0[:, ib:ib + BL], h0_d[:])
            nc.sync.dma_start(R1[:, ib:ib + BL], h1_d[:])
            nc.sync.dma_start(MB[:], mb_d[:])
            nc.sync.dma_start(YB[:], yb_d[:])
            NCHUNK = 8
            cw = (T * BL) // NCHUNK
            for ci in range(NCHUNK):
                cs = slice(ci * cw, (ci + 1) * cw)
                nc.sync.dma_start(UT[:, cs], u2_d[:, cs])
                nc.sync.dma_start(YT[:, cs], y_d[:, cs])

            def w(name):
                o, k, m = slices[name]
                return WT[0:k, o:o + m]

            # persistent PSUM banks (bufs=1 pools, tile per bank).
            # PN banks ping-pong by step parity so WAR signals to the PE
            # reach two steps back and stay off the critical chain.
            PG0 = pgp.tile([128, 2 * BL], f32)    # [ar0|az0]
            PG1 = pgp.tile([128, 2 * BL], f32)    # [ar1|az1]
            PN0ab = [pnp.tile([128, 4 * BL], f32, name=f"pn0{i}",
                              tag=f"pn0{i}") for i in range(2)]
            PN1ab = [pnp.tile([128, 4 * BL], f32, name=f"pn1{i}",
                              tag=f"pn1{i}") for i in range(2)]
            # each PN bank = [an_i | np | an_h | r]; the r slot keeps the
            # r-sigmoid output in PSUM (ACT PSUM access 172cy beats SBUF
            # 222cy, ~50ns off the on-chain sigmoid) and shares the bank's
            # step-parity ping-pong.

            # ring: slot(t) = [h_t | n_{t+1}]; h written by the DVE add at
            # step t, n_{t+1} written by ACT tanh at step t+1.
            def h_of(ring, t):   # h state AFTER step t (= input to t+1)
                base = (t % NBUF) * 2 * BL
                return ring[:, base:base + BL]

            def n_of(ring, t):   # n of step t, adjacent to h_{t-1}
                base = ((t - 1) % NBUF) * 2 * BL + BL
                return ring[:, base:base + BL]

            mm = nc.tensor.matmul
            AOP = mybir.AluOpType
            PQ1_prev = None

            for t in range(t_steps):
                tb = slice(t * BL, (t + 1) * BL)
                u2t = UT[:, tb]
                h0p = h_of(R0, t - 1)   # h0_{t-1} (bf16)
                h1p = h_of(R1, t - 1)
                PN0 = PN0ab[t % 2]
                PN1 = PN1ab[t % 2]
                np0 = PN0[:, BL:2 * BL]
                np1 = PN1[:, BL:2 * BL]

                # ---------- layer 0 gate matmuls ----------
                # On-chain contributions come from PQ1_prev (un-summed
                # halves of h1_{t-1}) so the DVE 'add' stays off-chain.
                mm(PG0[:, 0:BL], w("u2_r"), u2t, start=True, stop=False)
                mm(PG0[:, 0:BL], w("whh0_r"), h0p, start=False, stop=False)
                mm(PG0[:, BL:2 * BL], w("u2_z"), u2t, start=False, stop=False,
                   skip_group_check=True)
                mm(PG0[:, BL:2 * BL], w("whh0_z"), h0p, start=False,
                   stop=(t == 0), skip_group_check=True)
                mm(PN0[:, 0:BL], w("u2_n"), u2t, start=True, stop=False)
                mm(PN0[:, 2 * BL:3 * BL], w("whh0_n"), h0p, start=False,
                   stop=(t == 0), skip_group_check=True)
                if t >= 1:
                    qa, qb = PQ1_prev[:, 0:BL], PQ1_prev[:, BL:2 * BL]
                    mm(PG0[:, 0:BL], w("w0x_r"), qa, start=False, stop=False)
                    mm(PG0[:, BL:2 * BL], w("w0x_z"), qa, start=False,
                       stop=False, skip_group_check=True)
                    # n-gate can take the materialized h1 (an_i is consumed
                    # late enough); one MM instead of two
                    mm(PN0[:, 0:BL], w("w0x_n"), h1p, start=False, stop=True)
                    mm(PG0[:, 0:BL], w("w0x_r"), qb, start=False, stop=True)
                    mm(PG0[:, BL:2 * BL], w("w0x_z"), qb, start=False,
                       stop=True, skip_group_check=True)

                # ---------- layer 0 elementwise ----------
                # DVE carries ONLY the chain ops (P, np, qb); everything
                # off-chain (zc, qa, h-add) runs on the idle Pool engine so
                # chain ops never queue behind them in the DVE FIFO.
                SG0 = sgp.tile([128, 3 * BL], bf16, tag="sg0")  # [r|z|zc]
                nc.scalar.activation(SG0[:, 0:BL], PG0[:, 0:BL], AF.Sigmoid)
                nc.scalar.activation(SG0[:, BL:2 * BL], PG0[:, BL:2 * BL],
                                     AF.Sigmoid)
                P0 = smp.tile([128, BL], bf16, tag="p0")
                nc.vector.tensor_tensor(P0[:], SG0[:, 0:BL],
                                        PN0[:, 2 * BL:3 * BL],
                                        op=AOP.mult)          # r*an_h
                nc.vector.tensor_tensor(np0, PN0[:, 0:BL], P0[:],
                                        op=AOP.add)           # np
                nc.gpsimd.tensor_scalar(SG0[:, 2 * BL:3 * BL],
                                        SG0[:, BL:2 * BL], -1.0, 1.0,
                                        AOP.mult, AOP.add)    # zc = 1-z
                PQ0 = smp.tile([128, 2 * BL], bf16, tag="pq0")
                qa0, qb0 = PQ0[:, 0:BL], PQ0[:, BL:2 * BL]
                nc.gpsimd.tensor_tensor(qa0, SG0[:, BL:2 * BL],
                                        h_of(R0, t - 1),
                                        op=AOP.mult)          # z*h (early)
                nc.scalar.activation(n_of(R0, t), np0, AF.Tanh)
                nc.vector.tensor_tensor(qb0, SG0[:, 2 * BL:3 * BL],
                                        n_of(R0, t),
                                        op=AOP.mult)          # zc*n
                nc.gpsimd.tensor_tensor(h_of(R0, t), qa0, qb0,
                                        op=AOP.add)           # h0_t (ring)

                # ---------- layer 1 gate matmuls ----------
                mm(PG1[:, 0:BL], w("whh1_r"), h1p, start=True, stop=False)
                mm(PG1[:, BL:2 * BL], w("whh1_z"), h1p, start=False,
                   stop=False, skip_group_check=True)
                mm(PN1[:, 2 * BL:3 * BL], w("whh1_n"), h1p, start=True,
                   stop=False)
                mm(PG1[:, 0:BL], w("wih1_r"), qa0, start=False, stop=False)
                mm(PG1[:, BL:2 * BL], w("wih1_z"), qa0, start=False,
                   stop=False, skip_group_check=True)
                mm(PN1[:, 0:BL], w("wih1_n"), h_of(R0, t), start=False,
                   stop=True, skip_group_check=True)
                mm(PG1[:, 0:BL], w("wih1_r"), qb0, start=False, stop=True)
                pe_anchor = mm(PG1[:, BL:2 * BL], w("wih1_z"), qb0,
                               start=False, stop=True,
                               skip_group_check=True)

                # ---------- layer 1 elementwise ----------
                SG1 = sgp.tile([128, 3 * BL], bf16, tag="sg1")
                nc.scalar.activation(SG1[:, 0:BL], PG1[:, 0:BL], AF.Sigmoid)
                nc.scalar.activation(SG1[:, BL:2 * BL], PG1[:, BL:2 * BL],
                                     AF.Sigmoid)
                P1 = smp.tile([128, BL], bf16, tag="p1")
                nc.vector.tensor_tensor(P1[:], SG1[:, 0:BL],
                                        PN1[:, 2 * BL:3 * BL],
                                        op=AOP.mult)
                nc.vector.tensor_tensor(np1, PN1[:, 0:BL], P1[:],
                                        op=AOP.add)
                nc.gpsimd.tensor_scalar(SG1[:, 2 * BL:3 * BL],
                                        SG1[:, BL:2 * BL], -1.0, 1.0,
                                        AOP.mult, AOP.add)
                PQ1 = smp.tile([128, 2 * BL], bf16, tag="pq1")
                qa1, qb1 = PQ1[:, 0:BL], PQ1[:, BL:2 * BL]
                nc.gpsimd.tensor_tensor(qa1, SG1[:, BL:2 * BL],
                                        h_of(R1, t - 1),
                                        op=AOP.mult)          # z*h (early)
                act_anchor = nc.scalar.activation(n_of(R1, t), np1,
                                                  AF.Tanh)
                nc.vector.tensor_tensor(qb1, SG1[:, 2 * BL:3 * BL],
                                        n_of(R1, t),
                                        op=AOP.mult)
                dve_anchor = nc.gpsimd.tensor_tensor(
                    h_of(R1, t), qa1, qb1, op=AOP.add)        # h1_t (ring)
                PQ1_prev = PQ1

                def pin(inst, anchor, _on=False):
                    # order-only hint; measured slower than letting the
                    # scheduler place the (now small) menn bursts itself
                    if _on:
                        add_dep_helper(inst.ins, anchor.ins, sync=False,
                                       reason="menn after step chain ops")
                    return inst

                # ---------- batched menn + loss, spread over 10 phases ----
                # All phases reference only fully-past steps, so every
                # burst is ready the moment the PE/ACT reaches it in the
                # FIFO and is absorbed into idle windows (no chain stall).
                MH = MK // 2  # half-batch steps

                def h_batch(ring, s0, nsteps):
                    return ring[:, s0 * 2 * BL:(s0 + nsteps) * 2 * BL] \
                        .rearrange("p (k two) -> p k two",
                                   two=2 * BL)[:, :, 0:BL]

                if t >= MK:
                    ph = t % MK
                    k = t // MK - 1          # batch covering [t-16, t)@ph=0
                    hb0 = [k * MK, k * MK + MH]          # half start steps
                    hrg = [slice(0, MH * BL), slice(MH * BL, MK * BL)]
                    hbb = [slice(hb0[i] * BL, (hb0[i] + MH) * BL)
                           for i in range(2)]
                    if ph == 0:
                        menn_pm1 = pmp.tile([128, MK * BL], f32, tag="pm1")
                    if ph in (0, 3):         # mwu half A/B
                        half = ph // 3
                        pin(mm(menn_pm1[:, hrg[half]], w("mwu"),
                               UT[0:16, hbb[half]], start=(half == 0),
                               stop=False, skip_group_check=True), pe_anchor)
                    if ph in (1, 4):         # mw1h half A/B
                        half = ph // 4
                        pin(mm(menn_pm1[:, hrg[half]], w("mw1h"),
                               h_batch(R0, hb0[half] % NBUF, MH),
                               start=False, stop=False,
                               skip_group_check=True), pe_anchor)
                    if ph in (2, 5):         # mw1c half A/B
                        half = ph // 5
                        pin(mm(menn_pm1[:, hrg[half]], w("mw1c"),
                               h_batch(R1, hb0[half] % NBUF, MH),
                               start=False, stop=(half == 1),
                               skip_group_check=True), pe_anchor)
                    if ph == 6:
                        menn_m = smp.tile([128, MK * BL], bf16, tag="m")
                    if ph in (6, 7, 8, 9):   # relu quarters (fit idle slots)
                        q = ph - 6
                        qrg = slice(q * (MK // 4) * BL,
                                    (q + 1) * (MK // 4) * BL)
                        pin(nc.scalar.activation(menn_m[:, qrg],
                                                 menn_pm1[:, qrg],
                                                 AF.Relu, bias=MB[:]),
                            act_anchor)
                    if ph == 10:
                        menn_pmy = pmp.tile([Y, MK * BL], f32, tag="pmy")
                    if ph in (10, 12):       # mw32 half A/B
                        half = ph // 12
                        pin(mm(menn_pmy[:, hrg[half]], w("mw32"),
                               menn_m[:, hrg[half]], start=(half == 0),
                               stop=False, skip_group_check=True), pe_anchor)
                    if ph in (11, 13):       # negI half A/B
                        half = ph // 13
                        pin(mm(menn_pmy[:, hrg[half]], w("negI"),
                               YT[:, hbb[half]], start=False,
                               stop=(half == 1), skip_group_check=True),
                            pe_anchor)
                    if ph == 14:
                        menn_sq = smp.tile([Y, MK * BL], f32, tag="sq")
                    if ph in (14, 15):       # Square halves
                        half = ph - 14
                        pin(nc.scalar.activation(menn_sq[:, hrg[half]],
                                                 menn_pmy[:, hrg[half]],
                                                 AF.Square), act_anchor)
                    if ph == 15:             # reduce -> loss cols
                        for half in range(2):
                            col = 2 * k + half
                            nc.vector.reduce_sum(LOSS[:, col:col + 1],
                                                 menn_sq[:, hrg[half]],
                                                 axis=mybir.AxisListType.X)

            # flush the final menn batch
            k = t_steps // MK - 1
            menn_pm1 = pmp.tile([128, MK * BL], f32, tag="pm1")
            s0 = (k * MK) % NBUF
            bbk = slice(k * MK * BL, (k + 1) * MK * BL)
            mm(menn_pm1[:], w("mwu"), UT[0:16, bbk], start=True, stop=False)
            mm(menn_pm1[:], w("mw1h"), h_batch(R0, s0, MK), start=False,
               stop=False)
            mm(menn_pm1[:], w("mw1c"), h_batch(R1, s0, MK), start=False,
               stop=True)
            menn_m = smp.tile([128, MK * BL], bf16, tag="m")
            nc.scalar.activation(menn_m[:], menn_pm1[:], AF.Relu, bias=MB[:])
            menn_pmy = pmp.tile([Y, MK * BL], f32, tag="pmy")
            mm(menn_pmy[:], w("mw32"), menn_m[:], start=True, stop=False)
            mm(menn_pmy[:], w("negI"), YT[:, bbk], start=False, stop=True)
            MH = MK // 2
            menn_sq = smp.tile([Y, MK * BL], f32, tag="sq")
            nc.scalar.activation(menn_sq[:], menn_pmy[:], AF.Square)
            for half in range(2):
                rg = slice(half * MH * BL, (half + 1) * MH * BL)
                col = 2 * k + half
                nc.vector.reduce_sum(LOSS[:, col:col + 1], menn_sq[:, rg],
                                     axis=mybir.AxisListType.X)

            nc.sync.dma_start(out_d[:], LOSS[:])

    nc.finalize()
    return nc


_CACHE = {}


def kernel(**inputs) -> np.ndarray:
    from concourse.bass_utils import run_bass_kernel_spmd

    inputs = {k: np.asarray(v) for k, v in inputs.items()}
    comp = _compose_host(inputs)
    in_maps = _prep_core_inputs(inputs, comp)

    key = "graph"
    if key not in _CACHE:
        _CACHE[key] = build_graph(comp["slices"])
    nc = _CACHE[key]

    res = run_bass_kernel_spmd(nc, in_maps, core_ids=list(range(NCORES)))
    total = 0.0
    for r in res.results:
        total += np.asarray(r["out"], np.float64).sum()
    return np.float32(total)



# revision 19
# speedup vs baseline: 1.1591x; 1.1591x over previous
"""Trainium2 Bass kernel for a 2-layer GRU autoencoder RNN (B=256, T=1024, H=128).

Strategy
--------
- Data-parallel over batch: B=256 -> 32 rows per NeuronCore, 8 cores.
  The output is a scalar sum-of-squares loss, so each core computes a
  partial loss and the host sums the 8 partials. No collectives.
- The linear "dynn" head (x_t) is folded into composed weight matrices:
  the recurrent state is just (h0, h1); x_t is never materialized.
  u-dependent terms enter each step via a [u_t; u_{t-1}; 1] K=33 operand.
- Layout: features on partitions, batch (32) on the free dim.  Gate
  matmuls keep weights stationary ([K,128] lhsT slices of one packed
  SBUF tile) and stream the [128,32] state as the moving operand.
- The critical chain per GRU cell is sigmoid(r) -> r*an_h -> +an_i ->
  tanh -> zc*n -> gate matmuls; everything else (sigmoid(z), 1-z, z*h,
  the h=z*h+zc*n add, the readout MLP and the loss) is scheduled into
  the chain's idle engine slots.  The on-chain matmuls consume the
  un-summed (z*h, zc*n) halves so the DVE add stays off-chain; PN PSUM
  banks ping-pong by step parity to keep WAR event-semaphores out of
  the DVE FIFO.
- The readout MLP ("menn", with x_t folded in) and the loss are
  evaluated in 16-step batches (hidden matmuls, relu, output matmuls +
  Square-with-accum_out spread over three consecutive steps).
"""

import os
import sys
import numpy as np

sys.path.insert(0, "/opt/trn_rl_repo")

import ml_dtypes

BF16 = ml_dtypes.bfloat16

# problem constants (hardcoded per instructions)
B, T = 256, 1024
U, Z, Y, H = 16, 16, 16, 128
NCORES = 8
BL = B // NCORES  # 32 batch rows per core
NBUF = 32         # ring depth for [h|n] slots (32 so 16-step menn batches
                  # hit contiguous slot ranges)
MK = 16           # menn/loss batch size in steps


def _compose_host(inp):
    """All O(weight)-sized host-side algebra. Returns dict of np arrays."""
    f32 = np.float32
    Wih0, Whh0 = inp["Wih0"].astype(f32), inp["Whh0"].astype(f32)
    Wih1, Whh1 = inp["Wih1"].astype(f32), inp["Whh1"].astype(f32)
    dW1, db1 = inp["dW1"].astype(f32), inp["db1"].astype(f32)
    dW2, db2 = inp["dW2"].astype(f32), inp["db2"].astype(f32)
    mW1, mb1 = inp["mW1"].astype(f32), inp["mb1"].astype(f32)
    mW2, mb2 = inp["mW2"].astype(f32), inp["mb2"].astype(f32)
    mW3, mb3 = inp["mW3"].astype(f32), inp["mb3"].astype(f32)

    Wih0u, Wih0x = Wih0[:, :U], Wih0[:, U:]
    dW1u, dW1h = dW1[:, :U], dW1[:, U:]
    dWc = dW2 @ dW1h            # [16,128]
    dWpc = dW2 @ dW1u           # [16,16]
    cbias = db1 @ dW2.T + db2   # [16]

    W0x_eff = Wih0x @ dWc       # [384,128]
    W0upc = Wih0x @ dWpc        # [384,16]
    g0const = Wih0x @ cbias     # [384]

    mW1x, mW1h = mW1[:, :Z], mW1[:, Z:]
    mW1c = mW1x @ dWc           # [128,128]
    mWu = mW1x @ dWpc           # [128,16]
    mbias = mW1x @ cbias + mb1  # [128]
    mW32 = mW3 @ mW2            # [16,128]
    ybias = mW3 @ mb2 + mb3     # [16]

    # --- pack all stationary (lhsT) weights into one [128, ncols] tile ---
    # Each entry is lhsT = W.T with K on partitions (rows), M on columns.
    # u2-gate weights have K=33: rows 0:16 u_t part, 16:32 u_{t-1} part,
    # row 32 the constant (ones row of u2).
    slices = {}
    cols = []
    off = 0

    def add(name, mat_t):  # mat_t: [K, M] fp32
        nonlocal off
        k, m = mat_t.shape
        slices[name] = (off, k, m)
        cols.append(mat_t)
        off += m

    for g, sl in (("r", slice(0, H)), ("z", slice(H, 2 * H)),
                  ("n", slice(2 * H, 3 * H))):
        w_u2 = np.zeros((33, H), f32)
        w_u2[0:16] = Wih0u[sl].T
        w_u2[16:32] = W0upc[sl].T
        w_u2[32] = g0const[sl]
        add(f"u2_{g}", w_u2)
        add(f"whh0_{g}", Whh0[sl].T)        # [128,128]
        add(f"w0x_{g}", W0x_eff[sl].T)      # [128,128]
        add(f"wih1_{g}", Wih1[sl].T)        # [128,128]
        add(f"whh1_{g}", Whh1[sl].T)        # [128,128]
    add("mw1h", mW1h.T)                     # [128,128]
    add("mw1c", mW1c.T)                     # [128,128]
    add("mwu", mWu.T)                       # [16,128]
    add("mw32", mW32.T)                     # [128,16]
    negi = np.zeros((Y + 1, Y), f32)        # [-I; ybias] against [y; 1]
    negi[0:Y] = -np.eye(Y, dtype=f32)
    negi[Y] = ybias
    add("negI", negi)                       # [17,16]
    add("I128", np.eye(H, dtype=f32))       # [128,128]

    wpack = np.zeros((128, off), f32)
    o2 = 0
    for mat in cols:
        k, m = mat.shape
        wpack[:k, o2:o2 + m] = mat
        o2 += m

    return dict(wpack=wpack, slices=slices, mbias=mbias, ybias=ybias)


def _prep_core_inputs(inp, comp):
    """Per-core input arrays for the NEFF. Layout transforms only."""
    u = np.asarray(inp["u"], np.float32)    # [B, U, T]
    y = np.asarray(inp["y"], np.float32)    # [B, Y, T]
    h0 = np.asarray(inp["h0"], np.float32)  # [2, B, H]

    in_maps = []
    for c in range(NCORES):
        bs = slice(c * BL, (c + 1) * BL)
        uc = u[bs]                           # [BL, U, T]
        yc = y[bs]
        # u2: [33, T*BL]; column t*BL+b
        u2 = np.zeros((33, T * BL), np.float32)
        ut = np.transpose(uc, (1, 2, 0)).reshape(U, T * BL)   # [U, T*BL]
        u2[0:16] = ut
        u2[16:32, BL:] = ut[:, :-BL]        # u_{t-1}; zeros at t=0
        u2[32, BL:] = 1.0                   # const row; zero at t=0
        ysb = np.ones((Y + 1, T * BL), np.float32)            # [17, T*BL]
        ysb[0:Y] = np.transpose(yc, (1, 2, 0)).reshape(Y, T * BL)
        in_maps.append({
            "u2": u2.astype(BF16),
            "ysb": ysb.astype(BF16),
            "wpack": comp["wpack"].astype(BF16),
            "h0T": np.ascontiguousarray(h0[0, bs].T).astype(BF16),  # [128,BL]
            "h1T": np.ascontiguousarray(h0[1, bs].T).astype(BF16),
            "mbias": comp["mbias"].reshape(H, 1).astype(np.float32),
            "ybias": comp["ybias"].reshape(Y, 1).astype(np.float32),
        })
    return in_maps


def build_graph(slices, t_steps=T):
    """Build the Bass/Tile graph (one core's program; SPMD across 8)."""
    import concourse.bass as bass  # noqa
    import concourse.mybir as mybir
    import concourse.tile as tile
    from concourse import bacc
    from concourse.tile_rust import add_dep_helper

    f32 = mybir.dt.float32
    bf16 = mybir.dt.bfloat16
    AF = mybir.ActivationFunctionType

    nc = bacc.Bacc()
    wcols = max(o + m for (o, k, m) in slices.values())
    u2_d = nc.declare_dram_parameter("u2", [33, T * BL], bf16, isOutput=False)
    y_d = nc.declare_dram_parameter("ysb", [Y + 1, T * BL], bf16,
                                isOutput=False)
    w_d = nc.declare_dram_parameter("wpack", [128, wcols], bf16, isOutput=False)
    h0_d = nc.declare_dram_parameter("h0T", [H, BL], bf16, isOutput=False)
    h1_d = nc.declare_dram_parameter("h1T", [H, BL], bf16, isOutput=False)
    mb_d = nc.declare_dram_parameter("mbias", [H, 1], f32, isOutput=False)
    yb_d = nc.declare_dram_parameter("ybias", [Y, 1], f32, isOutput=False)
    out_d = nc.declare_dram_parameter("out", [Y, 2 * (T // MK)], f32,
                                      isOutput=True)

    with tile.TileContext(nc) as tc:
        # ---- persistent SBUF ----
        with (
            tc.tile_pool(name="resident", bufs=1) as rp,
            tc.tile_pool(name="sg", bufs=4) as sgp,
            tc.tile_pool(name="small", bufs=4) as smp,
            tc.tile_pool(name="pg", bufs=1, space="PSUM") as pgp,
            tc.tile_pool(name="pn", bufs=1, space="PSUM") as pnp,
            tc.tile_pool(name="pm", bufs=1, space="PSUM") as pmp,
        ):
            UT = rp.tile([33, T * BL], bf16)
            YT = rp.tile([Y + 1, T * BL], bf16)
            WT = rp.tile([128, wcols], bf16)
            MB = rp.tile([H, 1], f32)
            YB = rp.tile([Y, 1], f32)
            R0 = rp.tile([128, NBUF * 2 * BL], bf16)
            R1 = rp.tile([128, NBUF * 2 * BL], bf16)
            LOSS = rp.tile([Y, 2 * (T // MK)], f32)

            ib = (NBUF - 1) * 2 * BL  # slot(-1) h-half
            # small/critical tensors first, then the big streams in
            # chunks so early steps don't wait for the full transfer
            nc.sync.dma_start(WT[:], w_d[:])
            nc.sync.dma_start(R0[:, ib:ib + BL], h0_d[:])
            nc.sync.dma_start(R1[:, ib:ib + BL], h1_d[:])
            nc.sync.dma_start(MB[:], mb_d[:])
            nc.sync.dma_start(YB[:], yb_d[:])
            NCHUNK = 8
            cw = (T * BL) // NCHUNK
            for ci in range(NCHUNK):
                cs = slice(ci * cw, (ci + 1) * cw)
                nc.sync.dma_start(UT[:, cs], u2_d[:, cs])
                nc.sync.dma_start(YT[:, cs], y_d[:, cs])

            def w(name):
                o, k, m = slices[name]
                return WT[0:k, o:o + m]

            # persistent PSUM banks (bufs=1 pools, tile per bank).
            # PN banks ping-pong by step parity so WAR signals to the PE
            # reach two steps back and stay off the critical chain.
            PG0 = pgp.tile([128, 2 * BL], f32)    # [ar0|az0]
            PG1 = pgp.tile([128, 2 * BL], f32)    # [ar1|az1]
            PN0ab = [pnp.tile([128, 4 * BL], f32, name=f"pn0{i}",
                              tag=f"pn0{i}") for i in range(2)]
            PN1ab = [pnp.tile([128, 4 * BL], f32, name=f"pn1{i}",
                              tag=f"pn1{i}") for i in range(2)]
            # each PN bank = [an_i | np | an_h | r]; the r slot keeps the
            # r-sigmoid output in PSUM (ACT PSUM access 172cy beats SBUF
            # 222cy, ~50ns off the on-chain sigmoid) and shares the bank's
            # step-parity ping-pong.

            # ring: slot(t) = [h_t | n_{t+1}]; h written by the DVE add at
            # step t, n_{t+1} written by ACT tanh at step t+1.
            def h_of(ring, t):   # h state AFTER step t (= input to t+1)
                base = (t % NBUF) * 2 * BL
                return ring[:, base:base + BL]

            def n_of(ring, t):   # n of step t, adjacent to h_{t-1}
                base = ((t - 1) % NBUF) * 2 * BL + BL
                return ring[:, base:base + BL]

            mm = nc.tensor.matmul
            AOP = mybir.AluOpType
            PQ1_prev = None

            for t in range(t_steps):
                tb = slice(t * BL, (t + 1) * BL)
                u2t = UT[:, tb]
                h0p = h_of(R0, t - 1)   # h0_{t-1} (bf16)
                h1p = h_of(R1, t - 1)
                PN0 = PN0ab[t % 2]
                PN1 = PN1ab[t % 2]
                np0 = PN0[:, BL:2 * BL]
                np1 = PN1[:, BL:2 * BL]

                # ---------- layer 0 gate matmuls ----------
                # On-chain contributions come from PQ1_prev (un-summed
                # halves of h1_{t-1}) so the DVE 'add' stays off-chain.
                mm(PG0[:, 0:BL], w("u2_r"), u2t, start=True, stop=False)
                mm(PG0[:, 0:BL], w("whh0_r"), h0p, start=False, stop=False)
                mm(PG0[:, BL:2 * BL], w("u2_z"), u2t, start=False, stop=False,
                   skip_group_check=True)
                mm(PG0[:, BL:2 * BL], w("whh0_z"), h0p, start=False,
                   stop=(t == 0), skip_group_check=True)
                mm(PN0[:, 0:BL], w("u2_n"), u2t, start=True, stop=False)
                mm(PN0[:, 2 * BL:3 * BL], w("whh0_n"), h0p, start=False,
                   stop=(t == 0), skip_group_check=True)
                if t >= 1:
                    qa, qb = PQ1_prev[:, 0:BL], PQ1_prev[:, BL:2 * BL]
                    mm(PG0[:, 0:BL], w("w0x_r"), qa, start=False, stop=False)
                    mm(PG0[:, BL:2 * BL], w("w0x_z"), qa, start=False,
                       stop=False, skip_group_check=True)
                    # n-gate can take the materialized h1 (an_i is consumed
                    # late enough); one MM instead of two
                    mm(PN0[:, 0:BL], w("w0x_n"), h1p, start=False, stop=True)
                    mm(PG0[:, 0:BL], w("w0x_r"), qb, start=False, stop=True)
                    mm(PG0[:, BL:2 * BL], w("w0x_z"), qb, start=False,
                       stop=True, skip_group_check=True)

                # ---------- layer 0 elementwise ----------
                # DVE carries ONLY the chain ops (P, np, qb); everything
                # off-chain (zc, qa, h-add) runs on the idle Pool engine so
                # chain ops never queue behind them in the DVE FIFO.
                SG0 = sgp.tile([128, 3 * BL], bf16, tag="sg0")  # [r|z|zc]
                nc.scalar.activation(SG0[:, 0:BL], PG0[:, 0:BL], AF.Sigmoid)
                nc.scalar.activation(SG0[:, BL:2 * BL], PG0[:, BL:2 * BL],
                                     AF.Sigmoid)
                P0 = smp.tile([128, BL], bf16, tag="p0")
                nc.vector.tensor_tensor(P0[:], SG0[:, 0:BL],
                                        PN0[:, 2 * BL:3 * BL],
                                        op=AOP.mult)          # r*an_h
                nc.vector.tensor_tensor(np0, PN0[:, 0:BL], P0[:],
                                        op=AOP.add)           # np
                nc.vector.tensor_scalar(SG0[:, 2 * BL:3 * BL],
                                        SG0[:, BL:2 * BL], -1.0, 1.0,
                                        AOP.mult, AOP.add)    # zc = 1-z
                PQ0 = smp.tile([128, 2 * BL], bf16, tag="pq0")
                qa0, qb0 = PQ0[:, 0:BL], PQ0[:, BL:2 * BL]
                nc.vector.tensor_tensor(qa0, SG0[:, BL:2 * BL],
                                        h_of(R0, t - 1),
                                        op=AOP.mult)          # z*h (early)
                # the loss reduce for batch t//MK-2 rides the DVE idle
                # window while tanh0 runs (SQ finished a full step ago),
                # instead of blocking next step's chain ops at end-of-step
                if t >= 2 * MK and t % MK == 0:
                    kr = t // MK - 2
                    nc.vector.reduce_sum(LOSS[:, 2 * kr:2 * kr + 1],
                                         menn_sq_prev[:, 0:(MK // 2) * BL],
                                         axis=mybir.AxisListType.X)
                nc.scalar.activation(n_of(R0, t), np0, AF.Tanh)
                nc.vector.tensor_tensor(qb0, SG0[:, 2 * BL:3 * BL],
                                        n_of(R0, t),
                                        op=AOP.mult)          # zc*n
                nc.vector.tensor_tensor(h_of(R0, t), qa0, qb0,
                                        op=AOP.add)           # h0_t (ring)

                # ---------- layer 1 gate matmuls ----------
                mm(PG1[:, 0:BL], w("whh1_r"), h1p, start=True, stop=False)
                mm(PG1[:, BL:2 * BL], w("whh1_z"), h1p, start=False,
                   stop=False, skip_group_check=True)
                mm(PN1[:, 2 * BL:3 * BL], w("whh1_n"), h1p, start=True,
                   stop=False)
                mm(PG1[:, 0:BL], w("wih1_r"), qa0, start=False, stop=False)
                mm(PG1[:, BL:2 * BL], w("wih1_z"), qa0, start=False,
                   stop=False, skip_group_check=True)
                mm(PN1[:, 0:BL], w("wih1_n"), h_of(R0, t), start=False,
                   stop=True, skip_group_check=True)
                mm(PG1[:, 0:BL], w("wih1_r"), qb0, start=False, stop=True)
                pe_anchor = mm(PG1[:, BL:2 * BL], w("wih1_z"), qb0,
                               start=False, stop=True,
                               skip_group_check=True)

                # ---------- layer 1 elementwise ----------
                SG1 = sgp.tile([128, 3 * BL], bf16, tag="sg1")
                nc.scalar.activation(SG1[:, 0:BL], PG1[:, 0:BL], AF.Sigmoid)
                nc.scalar.activation(SG1[:, BL:2 * BL], PG1[:, BL:2 * BL],
                                     AF.Sigmoid)
                P1 = smp.tile([128, BL], bf16, tag="p1")
                nc.vector.tensor_tensor(P1[:], SG1[:, 0:BL],
                                        PN1[:, 2 * BL:3 * BL],
                                        op=AOP.mult)
                nc.vector.tensor_tensor(np1, PN1[:, 0:BL], P1[:],
                                        op=AOP.add)
                nc.vector.tensor_scalar(SG1[:, 2 * BL:3 * BL],
                                        SG1[:, BL:2 * BL], -1.0, 1.0,
                                        AOP.mult, AOP.add)
                PQ1 = smp.tile([128, 2 * BL], bf16, tag="pq1")
                qa1, qb1 = PQ1[:, 0:BL], PQ1[:, BL:2 * BL]
                nc.vector.tensor_tensor(qa1, SG1[:, BL:2 * BL],
                                        h_of(R1, t - 1),
                                        op=AOP.mult)          # z*h (early)
                if t >= 2 * MK and t % MK == 0:
                    kr = t // MK - 2
                    nc.vector.reduce_sum(
                        LOSS[:, 2 * kr + 1:2 * kr + 2],
                        menn_sq_prev[:, (MK // 2) * BL:MK * BL],
                        axis=mybir.AxisListType.X)
                act_anchor = nc.scalar.activation(n_of(R1, t), np1,
                                                  AF.Tanh)
                nc.vector.tensor_tensor(qb1, SG1[:, 2 * BL:3 * BL],
                                        n_of(R1, t),
                                        op=AOP.mult)
                dve_anchor = nc.vector.tensor_tensor(
                    h_of(R1, t), qa1, qb1, op=AOP.add)        # h1_t (ring)
                PQ1_prev = PQ1

                def pin(inst, anchor, _on=False):
                    # order-only hint; measured slower than letting the
                    # scheduler place the (now small) menn bursts itself
                    if _on:
                        add_dep_helper(inst.ins, anchor.ins, sync=False,
                                       reason="menn after step chain ops")
                    return inst

                # ---------- batched menn + loss, spread over 10 phases ----
                # All phases reference only fully-past steps, so every
                # burst is ready the moment the PE/ACT reaches it in the
                # FIFO and is absorbed into idle windows (no chain stall).
                MH = MK // 2  # half-batch steps

                def h_batch(ring, s0, nsteps):
                    return ring[:, s0 * 2 * BL:(s0 + nsteps) * 2 * BL] \
                        .rearrange("p (k two) -> p k two",
                                   two=2 * BL)[:, :, 0:BL]

                if t >= MK:
                    ph = t % MK
                    k = t // MK - 1          # batch covering [t-16, t)@ph=0
                    hb0 = [k * MK, k * MK + MH]          # half start steps
                    hrg = [slice(0, MH * BL), slice(MH * BL, MK * BL)]
                    hbb = [slice(hb0[i] * BL, (hb0[i] + MH) * BL)
                           for i in range(2)]
                    if ph == 0:
                        menn_pm1 = pmp.tile([128, MK * BL], f32, tag="pm1")
                    if ph in (0, 3):         # mwu half A/B
                        half = ph // 3
                        pin(mm(menn_pm1[:, hrg[half]], w("mwu"),
                               UT[0:16, hbb[half]], start=(half == 0),
                               stop=False, skip_group_check=True), pe_anchor)
                    if ph in (1, 4):         # mw1h half A/B
                        half = ph // 4
                        pin(mm(menn_pm1[:, hrg[half]], w("mw1h"),
                               h_batch(R0, hb0[half] % NBUF, MH),
                               start=False, stop=False,
                               skip_group_check=True), pe_anchor)
                    if ph in (2, 5):         # mw1c half A/B
                        half = ph // 5
                        pin(mm(menn_pm1[:, hrg[half]], w("mw1c"),
                               h_batch(R1, hb0[half] % NBUF, MH),
                               start=False, stop=(half == 1),
                               skip_group_check=True), pe_anchor)
                    if ph == 6:
                        menn_m = smp.tile([128, MK * BL], bf16, tag="m")
                    if ph in (6, 7, 8, 9):   # relu quarters (fit idle slots)
                        q = ph - 6
                        qrg = slice(q * (MK // 4) * BL,
                                    (q + 1) * (MK // 4) * BL)
                        pin(nc.scalar.activation(menn_m[:, qrg],
                                                 menn_pm1[:, qrg],
                                                 AF.Relu, bias=MB[:]),
                            act_anchor)
                    if ph == 10:
                        menn_pmy = pmp.tile([Y, MK * BL], f32, tag="pmy")
                    if ph in (10, 12):       # mw32 half A/B
                        half = ph // 12
                        pin(mm(menn_pmy[:, hrg[half]], w("mw32"),
                               menn_m[:, hrg[half]], start=(half == 0),
                               stop=False, skip_group_check=True), pe_anchor)
                    if ph in (11, 13):       # negI half A/B
                        half = ph // 13
                        pin(mm(menn_pmy[:, hrg[half]], w("negI"),
                               YT[:, hbb[half]], start=False,
                               stop=(half == 1), skip_group_check=True),
                            pe_anchor)
                    if ph == 14:
                        menn_sq = smp.tile([Y, MK * BL], f32, tag="sq")
                        menn_sq_prev = menn_sq
                    if ph in (14, 15):       # Square halves
                        half = ph - 14
                        pin(nc.scalar.activation(menn_sq[:, hrg[half]],
                                                 menn_pmy[:, hrg[half]],
                                                 AF.Square), act_anchor)
                    # (reduction issued next step, in the tanh idle slots)

            # flush: reduce the second-to-last batch (its in-loop slot at
            # ph==0 of the next window never comes), then the final batch
            MH = MK // 2
            kr = t_steps // MK - 2
            for half in range(2):
                rg = slice(half * MH * BL, (half + 1) * MH * BL)
                nc.vector.reduce_sum(LOSS[:, 2 * kr + half:2 * kr + half + 1],
                                     menn_sq_prev[:, rg],
                                     axis=mybir.AxisListType.X)
            k = t_steps // MK - 1
            menn_pm1 = pmp.tile([128, MK * BL], f32, tag="pm1")
            s0 = (k * MK) % NBUF
            bbk = slice(k * MK * BL, (k + 1) * MK * BL)
            mm(menn_pm1[:], w("mwu"), UT[0:16, bbk], start=True, stop=False)
            mm(menn_pm1[:], w("mw1h"), h_batch(R0, s0, MK), start=False,
               stop=False)
            mm(menn_pm1[:], w("mw1c"), h_batch(R1, s0, MK), start=False,
               stop=True)
            menn_m = smp.tile([128, MK * BL], bf16, tag="m")
            nc.scalar.activation(menn_m[:], menn_pm1[:], AF.Relu, bias=MB[:])
            menn_pmy = pmp.tile([Y, MK * BL], f32, tag="pmy")
            mm(menn_pmy[:], w("mw32"), menn_m[:], start=True, stop=False)
            mm(menn_pmy[:], w("negI"), YT[:, bbk], start=False, stop=True)
            MH = MK // 2
            menn_sq = smp.tile([Y, MK * BL], f32, tag="sq")
            nc.scalar.activation(menn_sq[:], menn_pmy[:], AF.Square)
            for half in range(2):
                rg = slice(half * MH * BL, (half + 1) * MH * BL)
                col = 2 * k + half
                nc.vector.reduce_sum(LOSS[:, col:col + 1], menn_sq[:, rg],
                                     axis=mybir.AxisListType.X)

            nc.sync.dma_start(out_d[:], LOSS[:])

    nc.finalize()
    return nc


_CACHE = {}


def kernel(**inputs) -> np.ndarray:
    from concourse.bass_utils import run_bass_kernel_spmd

    inputs = {k: np.asarray(v) for k, v in inputs.items()}
    comp = _compose_host(inputs)
    in_maps = _prep_core_inputs(inputs, comp)

    key = "graph"
    if key not in _CACHE:
        _CACHE[key] = build_graph(comp["slices"])
    nc = _CACHE[key]

    res = run_bass_kernel_spmd(nc, in_maps, core_ids=list(range(NCORES)))
    total = 0.0
    for r in res.results:
        total += np.asarray(r["out"], np.float64).sum()
    return np.float32(total)

